# revision 42
# baseline (speedup 1.0000x reference)
"""BertSelfAttention on 8 TRN2 NeuronCores (Bass/Tile).

Sharding: tensor-parallel over heads. Core c computes heads 2c, 2c+1
(output dims 128c : 128c+128). Each core receives the full hidden states
(pre-transposed and cast to bf16 on the host) plus its slice of the
Q/K/V projection weights, and produces its [B, 2, 65, S] slice of
unnormalized context (64 ctx rows + 1 softmax-denominator row per head,
feature-major); the host divides by the denominator, transposes to
[B, S, 128] and concatenates slices along the feature axis.

Per-core pipeline (B=4, S=2048, H=1024, NH=16, HD=64; 2 heads/core):
  1. proj:  QT,KT [128, S] (head dim on partitions) and V [S, 128]
            (seq on partitions) via PE matmuls over 8 contraction chunks.
            V rows are pre-scaled by exp(mask[key]) so the additive
            attention mask needs no per-key bias in the exp pass:
            exp(s+m) = exp(s)*exp(m), folded into V AND the ones column
            (so the denominator picks up exp(m) too).
  2. attn:  for each 512-wide query chunk, key blocks processed in PAIRS
            to keep the PE in one tile-mode for longer stretches:
              ST[k,q] for kb, kb+1 (two row-tiled 64-contraction matmuls
              each, heads packed into PE row halves - concurrent)
              P = exp(ST/8): ScalarE exp (psum -> sbuf bf16) for 3 of
              every 4 key blocks; the 4th uses a Schraudolph bitwise exp
              on the otherwise-idle DVE (round(184.665*(s/8)+C) written
              as int16 IS approx exp(s/8) when read back as bf16, C tuned
              for zero mean bias) - ScalarE throughput is the wall.
              OT[d,q] += [V*em | em]^T matmuls (M=65), 4 back-to-back.
            The ctx matmuls for a pair are emitted one iteration late
            (software pipelining) so the PE never waits on a fresh exp.
  3. out:   OT psum -> sbuf (DVE) -> DMA [B, 2, 65, S] fp32, host
            normalizes (divide by row 64) and transposes.
The bv bias is folded in on the host (softmax rows sum to one).
"""

import numpy as np
import ml_dtypes

import concourse.bass as bass
import concourse.mybir as mybir
import concourse.tile as tile
from concourse import bass_utils

B, S, H, NH, HD = 4, 2048, 1024, 16, 64
N_CORES = 8
DH = H // N_CORES          # 128 output dims per core (2 heads)
P = 128
QC = 512                   # query chunk (psum bank width in fp32)
NQC = S // QC              # 4
NKB = S // P               # 16 key blocks
NHC = H // P               # 8 contraction chunks for the projections
BF16 = mybir.dt.bfloat16
F32 = mybir.dt.float32

# experiment switches (set by compare_run.py; empty for the real kernel)
BUILD_FLAGS = {}


def _split_multi_waits(nc):
    # walrus in this container accepts at most ONE sync wait per
    # instruction; hoist extra waits onto preceding same-engine NOPs.
    n = 0
    for bb in nc.m.functions[0].blocks:
        new_insts = []
        for inst in bb.instructions:
            si = inst.sync_info
            if si is not None and si.on_wait:
                waits = list(si.on_wait)
                for w in waits[:-1]:
                    n += 1
                    new_insts.append(
                        mybir.InstNoOp(
                            name=f"waitsplit_{n}",
                            engine=inst.engine,
                            bass_nofuse=True,
                            sync_info=mybir.SyncInfo(on_wait=[w], on_update=[]),
                        )
                    )
                si.on_wait = waits[-1:]
            new_insts.append(inst)
        bb.instructions[:] = new_insts


def build_bass(reps=1):
    # Schraudolph-exp offload config: every _schr-th key block's exp runs
    # on the DVE instead of ScalarE (0 = off; default 4 = every 4th)
    if BUILD_FLAGS.get("schr2"):
        _schr = 2
    elif BUILD_FLAGS.get("schr8"):
        _schr = 8
    elif BUILD_FLAGS.get("schr0"):
        _schr = 0
    else:
        _schr = 4
    # which key block within the period gets the DVE exp: the FIRST of the
    # pair frees its score buffer via the (parallel) DVE rather than the
    # serialized ScalarE queue, unblocking the next group's score matmuls
    _schr_ph = _schr - 2 if BUILD_FLAGS.get("schrA") and _schr >= 2 else _schr - 1
    _schr_c = 16249.135 if BUILD_FLAGS.get("schrf") else 16248.636
    _pq512 = bool(BUILD_FLAGS.get("pq512"))
    _colslice = not BUILD_FLAGS.get("nocolslice")

    nc = bass.Bass("TRN2", target_bir_lowering=False, debug=False)
    xt = nc.dram_tensor("xt", [B, H, S], BF16, kind="ExternalInput").ap()
    wqt = nc.dram_tensor("wqt", [H, DH], BF16, kind="ExternalInput").ap()
    wkt = nc.dram_tensor("wkt", [H, DH], BF16, kind="ExternalInput").ap()
    wvt = nc.dram_tensor("wvt", [H, DH], BF16, kind="ExternalInput").ap()
    bqv = nc.dram_tensor("bqv", [DH], F32, kind="ExternalInput").ap()
    bkv = nc.dram_tensor("bkv", [DH], F32, kind="ExternalInput").ap()
    mask = nc.dram_tensor("mask", [B, S], F32, kind="ExternalInput").ap()
    outt = nc.dram_tensor("outt", [B, 2, HD + 1, S], F32, kind="ExternalOutput").ap()

    with tile.TileContext(nc) as tc:
        from contextlib import ExitStack

        with ExitStack() as ctx:
            consts = ctx.enter_context(tc.tile_pool(name="consts", bufs=1))
            xt_pool = ctx.enter_context(tc.tile_pool(name="xt", bufs=2))
            qkt_pool = ctx.enter_context(tc.tile_pool(name="qkt", bufs=2))
            # 3 bufs: batch b's von is still read by the carried-over ctx
            # while start_b(b+2) already writes its denominator columns
            von_pool = ctx.enter_context(tc.tile_pool(name="von", bufs=3))
            ex_pool = ctx.enter_context(
                tc.tile_pool(name="ex", bufs=6 if BUILD_FLAGS.get("ex6") else 4)
            )
            ob_pool = ctx.enter_context(tc.tile_pool(name="ob", bufs=2))
            mask_pool = ctx.enter_context(tc.tile_pool(name="maskp", bufs=2))
            em_pool = ctx.enter_context(tc.tile_pool(name="emp", bufs=2))
            # PSUM budget (8 banks): stp 2x2 + ot 2 + misc 2
            ps_misc = ctx.enter_context(tc.tile_pool(name="ps_misc", bufs=2, space="PSUM"))
            ps_st = ctx.enter_context(tc.tile_pool(name="ps_st", bufs=2, space="PSUM"))
            ps_ot = ctx.enter_context(tc.tile_pool(name="ps_ot", bufs=2, space="PSUM"))

            # constants (DMAs deferred until after batch 0's first xt
            # column slices so they don't delay the critical first slice)
            wq_sb = consts.tile([P, NHC, DH], BF16, name="wq_sb")
            wk_sb = consts.tile([P, NHC, DH], BF16, name="wk_sb")
            wv_sb = consts.tile([P, NHC, DH], BF16, name="wv_sb")
            bq_sb = consts.tile([P, 1], F32, name="bq_sb")
            bk_sb = consts.tile([P, 1], F32, name="bk_sb")

            def load_consts():
                _weng = nc.scalar if BUILD_FLAGS.get("dmasplit") else nc.sync
                _weng.dma_start(wk_sb[:], wkt.rearrange("(hc p) d -> p hc d", p=P))
                _weng.dma_start(wq_sb[:], wqt.rearrange("(hc p) d -> p hc d", p=P))
                _weng.dma_start(wv_sb[:], wvt.rearrange("(hc p) d -> p hc d", p=P))
                _weng.dma_start(bq_sb[:], bqv[:, None])
                _weng.dma_start(bk_sb[:], bkv[:, None])

            def start_b(b, prologue=False, after_first_slice=None):
                """Allocate per-batch tiles, issue input DMAs, and build the
                list of projection work units (each ~8 matmuls + evac)."""
                st = {}
                st["xt"] = xt_pool.tile([P, NHC, S], BF16, name="xt_b", tag="xt_b")
                xr = xt[b].rearrange("(hc p) s -> p hc s", p=P)
                if prologue and _colslice:
                    # batch 0 gates the whole pipeline: every projection
                    # unit contracts over ALL hc chunks but reads only a
                    # 256-wide column slice, so land complete column
                    # slices first and attention can start after slice 1
                    # instead of after the full 4MB.
                    CW = S // NHC  # 256
                    for j in range(NHC):
                        jsl = slice(j * CW, (j + 1) * CW)
                        nc.sync.dma_start(st["xt"][:, :, jsl], xr[:, :, jsl])
                        if j == 0 and after_first_slice is not None:
                            # weights slot in right behind the critical
                            # first column slice
                            after_first_slice()
                else:
                    if after_first_slice is not None:
                        after_first_slice()
                    for hc in range(NHC):
                        # per-chunk DMAs so the first projection matmuls
                        # can start before the whole 4MB slice has landed
                        eng = (
                            nc.scalar
                            if (prologue and BUILD_FLAGS.get("dmasplit") and hc % 2)
                            else nc.sync
                        )
                        eng.dma_start(st["xt"][:, hc, :], xr[:, hc, :])
                st["mask"] = mask_pool.tile([P, NKB], F32, name="mask_b", tag="mask_b")
                nc.sync.dma_start(
                    st["mask"][:], mask[b].rearrange("(kb p) -> p kb", p=P)
                )
                st["em"] = em_pool.tile([P, NKB], F32, name="em_b", tag="em_b")
                nc.scalar.activation(
                    st["em"][:], st["mask"][:], mybir.ActivationFunctionType.Exp
                )
                st["qt"] = qkt_pool.tile([P, S], BF16, name="qt", tag="qt")
                st["kt"] = qkt_pool.tile([P, S], BF16, name="kt", tag="kt")
                st["von"] = von_pool.tile(
                    [P, NKB, 2 * (HD + 1)], BF16, name="von", tag="von"
                )
                # denominator columns = exp(mask) per key
                nc.vector.tensor_copy(st["von"][:, :, HD:HD + 1], st["em"][:, :, None])
                nc.vector.tensor_copy(
                    st["von"][:, :, 2 * HD + 1:2 * HD + 2], st["em"][:, :, None]
                )
                # Unit order matters: attention on (b, qc=0) needs all kt
                # chunks, qt chunk 0, and the first few von blocks. pq/pk
                # units are 256-wide halves so injected bursts stay short.
                if prologue and _colslice:
                    # consumption order: group g of qc 0 needs only pk g
                    # (kt cols 256g:256g+256), qt chunk 0 and von pair g;
                    # each pk/pv lands one group ahead of its consumer as
                    # its column slice arrives.
                    st["units"] = (
                        [("pk", 0), ("pq", 0), ("pv", 0), ("pv", 1), ("pq", 1)]
                        + [u for g in range(1, 2 * NQC)
                           for u in (("pk", g), ("pv", 2 * g), ("pv", 2 * g + 1))]
                        + [("pq", i) for i in range(2, 2 * NQC)]
                    )
                    st["n_prologue"] = 5
                elif _pq512:
                    st["units"] = (
                        [("pk", i) for i in range(NQC)]
                        + [("pq", 0)]
                        + [("pv", kb) for kb in range(4)]
                        + [("pq", 1), ("pv", 4), ("pv", 5), ("pq", 2),
                           ("pv", 6), ("pv", 7), ("pq", 3)]
                        + [("pv", kb) for kb in range(8, NKB)]
                    )
                    st["n_prologue"] = 9
                else:
                    st["units"] = (
                        [("pk", i) for i in range(2 * NQC)]
                        + [("pq", 0), ("pq", 1)]
                        + [("pv", kb) for kb in range(4)]
                        + [("pq", 2), ("pq", 3), ("pv", 4), ("pv", 5), ("pq", 4),
                           ("pq", 5), ("pv", 6), ("pv", 7), ("pq", 6), ("pq", 7)]
                        + [("pv", kb) for kb in range(8, NKB)]
                    )
                    st["n_prologue"] = 14
                return st

            HQ = QC if _pq512 else QC // 2  # pq/pk unit width
            NG_UNITS = 2 * NQC  # pk units == key-block pairs per qc

            def emit_unit(st, unit):
                kind, idx = unit
                if kind in ("pq", "pk"):
                    w_sb = wq_sb if kind == "pq" else wk_sb
                    b_sb = bq_sb if kind == "pq" else bk_sb
                    dest = st["qt"] if kind == "pq" else st["kt"]
                    pp = ps_misc.tile([P, HQ], F32, name=kind, tag="misc",
                                      padded_shape=[P, QC])
                    for h in range(NHC):
                        nc.tensor.matmul(
                            pp[:],
                            lhsT=w_sb[:, h, :],
                            rhs=st["xt"][:, h, idx * HQ:(idx + 1) * HQ],
                            start=(h == 0),
                            stop=(h == NHC - 1),
                        )
                    nc.vector.tensor_tensor(
                        dest[:, idx * HQ:(idx + 1) * HQ],
                        pp[:],
                        b_sb[:].to_broadcast((P, HQ)),
                        mybir.AluOpType.add,
                    )
                else:  # pv: V block idx in [s, d] layout, scaled by exp(mask)
                    pv = ps_misc.tile([P, P], F32, name="pv", tag="misc",
                                      padded_shape=[P, QC])
                    for h in range(NHC):
                        nc.tensor.matmul(
                            pv[:],
                            lhsT=st["xt"][:, h, idx * P:(idx + 1) * P],
                            rhs=wv_sb[:, h, :],
                            start=(h == 0),
                            stop=(h == NHC - 1),
                        )
                    nc.vector.tensor_copy(st["von"][:, idx, 0:HD], pv[:, 0:HD])
                    nc.vector.tensor_copy(
                        st["von"][:, idx, HD + 1:2 * HD + 1], pv[:, HD:2 * HD]
                    )
                    nc.vector.tensor_scalar_mul(
                        st["von"][:, idx, 0:HD], st["von"][:, idx, 0:HD],
                        st["em"][:, idx:idx + 1],
                    )
                    nc.vector.tensor_scalar_mul(
                        st["von"][:, idx, HD + 1:2 * HD + 1],
                        st["von"][:, idx, HD + 1:2 * HD + 1],
                        st["em"][:, idx:idx + 1],
                    )

            seq = [b for _ in range(reps) for b in range(B)]
            state = {}
            # prologue for the first batch: enough projections to start
            # attention (all kt chunks, qt chunk 0, first 4 V blocks);
            # the rest is injected into the first attention qc's k-loop.
            state[0] = start_b(seq[0], prologue=True, after_first_slice=load_consts)
            _np0 = state[0]["n_prologue"]
            for u in state[0]["units"][:_np0]:
                emit_unit(state[0], u)
            own_pending = list(state[0]["units"][_np0:])

            NG = NKB // 2  # key-block pairs per query chunk

            def emit_ctx(pend):
                p_kbs, p_exs, p_ot0, p_ot1, p_von, _, _ = pend
                for kb, ex in zip(p_kbs, p_exs):
                    nc.tensor.matmul(
                        p_ot0[0:HD + 1, :],
                        lhsT=p_von[:, kb, 0:HD + 1],
                        rhs=ex[:, 0:QC],
                        start=(kb == 0),
                        stop=(kb == NKB - 1),
                    )
                    nc.tensor.matmul(
                        p_ot1[0:HD + 1, :],
                        lhsT=p_von[:, kb, HD + 1:2 * HD + 2],
                        rhs=ex[:, QC:2 * QC],
                        start=(kb == 0),
                        stop=(kb == NKB - 1),
                    )

            def emit_evac(pend):
                # unnormalized context + denominators out; the host
                # divides and transposes.
                _, _, p_ot0, p_ot1, _, p_b, p_qc = pend
                qsl = slice(p_qc * QC, (p_qc + 1) * QC)
                ob = ob_pool.tile([HD + 1, 2, QC], F32, name="ob")
                nc.vector.tensor_copy(ob[:, 0, :], p_ot0[0:HD + 1, :])
                nc.vector.tensor_copy(ob[:, 1, :], p_ot1[0:HD + 1, :])
                nc.sync.dma_start(
                    outt[p_b][:, :, qsl].rearrange("h d q -> d h q"),
                    ob[:],
                )

            # ---- attention (with projection work injected) ----
            # Software-pipelined across the whole sequence of batches: the
            # ctx matmuls for a key-block pair are emitted one iteration
            # late so the PE never sits on a just-issued exp.
            pend = None  # (kbs, exs, ot0, ot1, von, b, qc)
            for pos, b in enumerate(seq):
                stt = state[pos]
                qt = stt["qt"]
                kt = stt["kt"]
                von = stt["von"]
                if pos + 1 < len(seq):
                    state[pos + 1] = start_b(seq[pos + 1])
                    next_units = list(state[pos + 1]["units"])
                else:
                    next_units = []
                state.pop(pos - 1, None)

                inj_i = 0
                for qc in range(NQC):
                    qsl = slice(qc * QC, (qc + 1) * QC)
                    ot0 = ps_ot.tile([P, QC], F32, name="ot0", tag="ot")
                    ot1 = ps_ot.tile([P, QC], F32, name="ot1", tag="ot")
                    for g in range(NG):
                        it = qc * NG + g
                        kbs = (2 * g, 2 * g + 1)
                        stps = []
                        for kb in kbs:
                            stp = ps_st.tile([P, 2 * QC], F32, name="stp")
                            stps.append(stp)
                            nc.tensor.matmul(
                                stp[:, 0:QC],
                                lhsT=kt[0:HD, kb * P:(kb + 1) * P],
                                rhs=qt[0:HD, qsl],
                                start=True,
                                stop=True,
                            )
                            nc.tensor.matmul(
                                stp[:, QC:2 * QC],
                                lhsT=kt[HD:2 * HD, kb * P:(kb + 1) * P],
                                rhs=qt[HD:2 * HD, qsl],
                                start=True,
                                stop=True,
                            )
                        exs = []
                        for kb, stp in zip(kbs, stps):
                            ex = ex_pool.tile([P, 2 * QC], BF16, name="ex")
                            exs.append(ex)
                            if _schr and kb % _schr == _schr_ph:
                                # Schraudolph exp on the DVE: bf16's bit
                                # pattern is linear in log2(value), so
                                # round(184.665*(s/8) + C) stored as int16
                                # IS approximately exp(s/8) when the same
                                # bytes are read back as bf16. C is tuned
                                # so the mean multiplicative bias over the
                                # score distribution is 1. Offloading a
                                # subset of key blocks relieves ScalarE,
                                # the throughput wall.
                                nc.vector.tensor_scalar(
                                    ex[:].bitcast(mybir.dt.int16),
                                    stp[:],
                                    184.66496 / np.sqrt(HD),
                                    _schr_c,
                                    mybir.AluOpType.mult,
                                    mybir.AluOpType.add,
                                )
                            else:
                                nc.scalar.activation(
                                    ex[:],
                                    stp[:],
                                    mybir.ActivationFunctionType.Exp,
                                    scale=1.0 / np.sqrt(HD),
                                )
                        if pend is not None:
                            emit_ctx(pend)
                            if pend[6] is not None and pend[0][1] == NKB - 1:
                                emit_evac(pend)
                        # proj work lands here: same PE tile-mode as ctx,
                        # fills the PE while ScalarE works on this group
                        if own_pending:
                            for u in own_pending[:3]:
                                emit_unit(stt, u)
                            del own_pending[:3]
                        elif next_units and inj_i < len(next_units):
                            target = min(
                                len(next_units),
                                it * len(next_units) // (NQC * NG - 8) + 1,
                            )
                            while inj_i < target:
                                emit_unit(state[pos + 1], next_units[inj_i])
                                inj_i += 1
                        pend = (kbs, exs, ot0, ot1, von, b, qc)
            # flush the last group of the last batch
            emit_ctx(pend)
            emit_evac(pend)
    _split_multi_waits(nc)
    return nc


def host_prep(hidden_states, attention_mask, Wq, bq, Wk, bk, Wv, bv):
    xt_np = np.ascontiguousarray(
        np.asarray(hidden_states).transpose(0, 2, 1)
    ).astype(ml_dtypes.bfloat16)
    mask_np = np.ascontiguousarray(
        np.asarray(attention_mask).reshape(B, S)
    ).astype(np.float32)
    in_maps = []
    for c in range(N_CORES):
        dsl = slice(c * DH, (c + 1) * DH)
        in_maps.append(
            {
                "xt": xt_np,
                "wqt": np.ascontiguousarray(np.asarray(Wq)[dsl, :].T).astype(ml_dtypes.bfloat16),
                "wkt": np.ascontiguousarray(np.asarray(Wk)[dsl, :].T).astype(ml_dtypes.bfloat16),
                "wvt": np.ascontiguousarray(np.asarray(Wv)[dsl, :].T).astype(ml_dtypes.bfloat16),
                "bqv": np.ascontiguousarray(np.asarray(bq)[dsl]).astype(np.float32),
                "bkv": np.ascontiguousarray(np.asarray(bk)[dsl]).astype(np.float32),
                "mask": mask_np,
            }
        )
    return in_maps


def gather(results, bv):
    out = np.empty((B, S, H), np.float32)
    for c in range(N_CORES):
        o = results[c]["outt"]  # [B, 2, 65, S] unnormalized + denominator
        for h in range(2):
            ctx = o[:, h, 0:HD, :] / o[:, h, HD:HD + 1, :]
            out[:, :, c * DH + h * HD:c * DH + (h + 1) * HD] = ctx.transpose(0, 2, 1)
    # bv folded on the host: softmax rows sum to 1, so ctx(V+bv)=ctx(V)+bv
    out += np.asarray(bv).astype(np.float32)[None, None, :]
    return out


def make_runner(nc, in_maps):
    """Build a reusable jitted 8-core runner for `nc` (mirrors
    bass2jax.run_bass_via_pjrt's multi-core path, but keeps the jitted
    callable so repeated executions don't re-lower)."""
    import jax
    from jax.sharding import Mesh, NamedSharding, PartitionSpec
    from jax.experimental.shard_map import shard_map
    from concourse import bass2jax

    bass2jax.install_neuronx_cc_hook()
    partition_name = nc.partition_id_tensor.name if nc.partition_id_tensor else None
    in_names, out_names, out_avals, zero_outs = [], [], [], []
    for alloc in nc.m.functions[0].allocations:
        if not isinstance(alloc, mybir.MemoryLocationSet):
            continue
        name = alloc.memorylocations[0].name
        if alloc.kind == "ExternalInput":
            if name != partition_name:
                in_names.append(name)
        elif alloc.kind == "ExternalOutput":
            out_names.append(name)
            shape = tuple(alloc.tensor_shape)
            dtype = mybir.dt.np(alloc.dtype)
            out_avals.append(jax.core.ShapedArray(shape, dtype))
            zero_outs.append(np.zeros(shape, dtype))
    n_params = len(in_names)
    n_outs = len(out_avals)
    all_in = list(in_names) + list(out_names)
    if partition_name is not None:
        all_in.append(partition_name)

    def _body(*args):
        operands = list(args)
        if partition_name is not None:
            operands.append(bass2jax.partition_id_tensor())
        outs = bass2jax._bass_exec_p.bind(
            *operands,
            out_avals=tuple(out_avals),
            in_names=tuple(all_in),
            out_names=tuple(out_names),
            lowering_input_output_aliases=(),
            sim_require_finite=True,
            sim_require_nnan=True,
            nc=nc,
        )
        return tuple(outs)

    devices = jax.devices()[:N_CORES]
    mesh = Mesh(np.asarray(devices), ("core",))
    sharded = jax.jit(
        shard_map(
            _body,
            mesh=mesh,
            in_specs=(PartitionSpec("core"),) * (n_params + n_outs),
            out_specs=(PartitionSpec("core"),) * n_outs,
            check_rep=False,
        ),
        keep_unused=True,
    )
    per_core = [[np.asarray(m[name]) for name in in_names[:n_params]] for m in in_maps]
    concat_in = [
        np.concatenate([per_core[c][i] for c in range(N_CORES)], axis=0)
        for i in range(n_params)
    ]
    concat_zeros = [
        np.zeros((N_CORES * z.shape[0], *z.shape[1:]), z.dtype) for z in zero_outs
    ]
    sh = NamedSharding(mesh, PartitionSpec("core"))
    args_dev = [jax.device_put(a, sh) for a in concat_in] + [
        jax.device_put(a, sh) for a in concat_zeros
    ]

    def run():
        import jax as _jax

        outs = sharded(*args_dev)
        _jax.block_until_ready(outs)
        return [
            {
                name: np.asarray(outs[i]).reshape(N_CORES, *out_avals[i].shape)[c]
                for i, name in enumerate(out_names)
            }
            for c in range(N_CORES)
        ]

    def run_nofetch():
        import jax as _jax

        outs = sharded(*args_dev)
        _jax.block_until_ready(outs)

    run.nofetch = run_nofetch
    return run


def kernel(hidden_states, attention_mask, Wq, bq, Wk, bk, Wv, bv):
    in_maps = host_prep(hidden_states, attention_mask, Wq, bq, Wk, bk, Wv, bv)
    nc = build_bass()
    res = bass_utils.run_bass_kernel_spmd(nc, in_maps, core_ids=list(range(N_CORES)))
    return gather(res.results, bv)
